# revision 1
# baseline (speedup 1.0000x reference)
"""Causal multi-head attention on 8 TRN2 NeuronCores.

Problem: B=4, T=2048, d_model=1024, 16 heads x 64. out = softmax(causal(QK^T)/8) V Wo.

Sharding (tensor-parallel heads x data-parallel batch):
  core c -> batch b = c//2, head group g = c%2 (8 heads each).
  Each core computes a partial output  z_g[b] @ Wo[g] : [2048, 1024];
  host sums the two head-group partials per batch.

Per-core kernel (all matmuls in fp32r = fp32 data at full PE rate):
  - host passes x[b]^T (so d_model is the SBUF partition dim everywhere)
  - fused single pass over 4 query chunks of 512:
      proj (Q^T,K^T,V for the chunk) -> causal flash-style attention over
      k-blocks of 128 (scores pairs of heads via tile_position row-split,
      exp on ACT, V-augmented-with-ones matmul accumulates both z^T and the
      softmax denominator) -> divide -> output projection -> DMA out.
"""
import numpy as np

import concourse.bass as bass
import concourse.tile as tile
import concourse.mybir as mybir
from concourse.vector_clock import ScopedClock
from concourse.bass_utils import run_bass_kernel_spmd

D_MODEL = 1024
D_HEAD = 64
B = 4
T = 2048
H = 8              # heads per core
HG = H * D_HEAD    # 512 head-dim columns per core
TCH = 512          # q/t chunk
NCH = T // TCH     # 4
NDM = D_MODEL // 128  # 8 d_model chunks

F32R = mybir.dt.float32r
F32 = mybir.dt.float32
BF16 = mybir.dt.bfloat16
AF = mybir.ActivationFunctionType


class _TC(tile.TileContext):
    """TileContext whose tail drain carries no sem waits (this walrus build
    rejects >1 sync wait per instruction and any wait on a Drain)."""

    def _drain_and_barrier(self, tick_clock, wait_clock):
        drain_inst = self.nc.sync.drain()
        wait_clock.add_sem_waits(
            drain_inst.ins, ScopedClock({None: tick_clock.global_clock})
        )
        si = drain_inst.ins.sync_info
        waits = list(si.on_wait) if si is not None else []
        if waits:
            drain_inst.ins.sync_info = mybir.SyncInfo(
                on_wait=[], on_update=list(si.on_update)
            )
            for w in waits:
                nop = self.nc.sync.nop(nofuse=True)
                nop.ins.sync_info = mybir.SyncInfo(on_wait=[w], on_update=[])
        self.nc.all_engine_barrier()
        popped = self.nc._tile_sem_poison_stack.pop()
        assert popped is self._sem_poison
        self.nc.clear_and_free_semaphores(list(self.sems.allocated().values()))
        self.nc.all_engine_barrier()


def _split_multi_waits(nc):
    """Move all-but-one sem wait of every instruction onto same-engine NOPs."""
    cnt = 0
    for f in nc.m.functions:
        for b in f.blocks:
            new = []
            for inst in b.instructions:
                si = inst.sync_info
                if si is not None and si.on_wait is not None:
                    waits = list(si.on_wait)
                    max_keep = 0 if inst.opcode == "Drain" else 1
                    if len(waits) > max_keep:
                        keep = waits[len(waits) - max_keep:] if max_keep else []
                        spill = waits[: len(waits) - max_keep]
                        for w in spill:
                            nop = mybir.InstNoOp(
                                name=f"I-wsplit-{cnt}", engine=inst.engine,
                                ins=[], outs=[],
                            )
                            nop.sync_info = mybir.SyncInfo(
                                on_wait=[w], on_update=[]
                            )
                            new.append(nop)
                            cnt += 1
                        inst.sync_info = mybir.SyncInfo(
                            on_wait=keep, on_update=list(si.on_update)
                        )
                new.append(inst)
            b.instructions = new
    return cnt


def _build():
    nc = bass.Bass("TRN2", target_bir_lowering=False)
    xT = nc.dram_tensor("xT", (D_MODEL, T), F32R, kind="ExternalInput")
    wq = nc.dram_tensor("wq", (D_MODEL, HG), F32R, kind="ExternalInput")
    wk = nc.dram_tensor("wk", (D_MODEL, HG), F32R, kind="ExternalInput")
    wv = nc.dram_tensor("wv", (D_MODEL, HG), F32R, kind="ExternalInput")
    wo = nc.dram_tensor("wo", (HG, D_MODEL), F32R, kind="ExternalInput")
    tri = nc.dram_tensor("tri", (128, 128), BF16, kind="ExternalInput")
    ones1 = nc.dram_tensor("ones1", (1, 64), F32R, kind="ExternalInput")
    vones = nc.dram_tensor("vones", (128, T // 128, H, 1), BF16,
                           kind="ExternalInput")
    out = nc.dram_tensor("out", (T, D_MODEL), F32, kind="ExternalOutput")

    from contextlib import ExitStack
    with _TC(nc) as tc, ExitStack() as ctx:
        consts = ctx.enter_context(tc.tile_pool(name="consts", bufs=1))
        xs_pool = ctx.enter_context(tc.tile_pool(name="xs", bufs=3))
        kt_pool = ctx.enter_context(tc.tile_pool(name="kt", bufs=1))
        v_pool = ctx.enter_context(tc.tile_pool(name="v", bufs=1))
        qt_pool = ctx.enter_context(tc.tile_pool(name="qt", bufs=3))
        zt_pool = ctx.enter_context(tc.tile_pool(name="zt", bufs=2))
        et_pool = ctx.enter_context(tc.tile_pool(name="et", bufs=4))
        sm_pool = ctx.enter_context(tc.tile_pool(name="sm", bufs=2))
        rb_pool = ctx.enter_context(tc.tile_pool(name="rb", bufs=2))
        ou_pool = ctx.enter_context(tc.tile_pool(name="ou", bufs=2))
        ps_s = ctx.enter_context(tc.tile_pool(name="ps_s", bufs=2, space="PSUM"))
        ps_u = ctx.enter_context(tc.tile_pool(name="ps_u", bufs=2, space="PSUM"))
        ps_w = ctx.enter_context(tc.tile_pool(name="ps_w", bufs=2, space="PSUM"))

        xT_r = xT.ap().rearrange("(c p) t -> p c t", p=128)

        # resident weights / constants (wq/wk + first x chunk lead: they gate
        # the first matmuls; wo is not needed until the first out-proj)
        wq_sb = consts.tile([128, NDM, HG], F32R)
        xs0 = xs_pool.tile([128, NDM, TCH], F32R, name="xs", tag="xs")
        wk_sb = consts.tile([128, NDM, HG], F32R)
        wv_sb = consts.tile([128, NDM, HG], F32R)
        wq_r = wq.ap().rearrange("(c p) n -> p c n", p=128)
        wk_r = wk.ap().rearrange("(c p) n -> p c n", p=128)
        for c in range(NDM):
            nc.sync.dma_start(out=wq_sb[:, c, :], in_=wq_r[:, c, :])
            nc.sync.dma_start(out=wk_sb[:, c, :], in_=wk_r[:, c, :])
            nc.sync.dma_start(out=xs0[:, c, :], in_=xT_r[:, c, 0:TCH])
        nc.sync.dma_start(out=wv_sb, in_=wv.ap().rearrange("(c p) n -> p c n", p=128))
        tri_sb = consts.tile([128, 128], BF16)
        nc.sync.dma_start(out=tri_sb, in_=tri.ap())
        ones_sb = consts.tile([1, 64], F32R)
        nc.sync.dma_start(out=ones_sb, in_=ones1.ap())
        wo_sb = consts.tile([128, HG // 128, D_MODEL], F32R)
        nc.sync.dma_start(out=wo_sb, in_=wo.ap().rearrange("(c p) n -> p c n", p=128))
        # per-chunk K^T tiles [pair-packed 128, pair, t-in-chunk] and V tiles
        # (V has a ones column so row 64 of U accumulates the denominator)
        kt_tiles = [kt_pool.tile([128, 4, TCH], BF16, name=f"kt{i}", tag=f"kt{i}")
                    for i in range(NCH)]
        v_tiles = [v_pool.tile([128, 4, H, D_HEAD + 1], BF16, name=f"v{i}",
                               tag=f"v{i}") for i in range(NCH)]
        vo_r = vones.ap().rearrange("p (a b) h o -> p a b h o", b=4)
        for i in range(NCH):
            nc.sync.dma_start(out=v_tiles[i][:, :, :, D_HEAD:], in_=vo_r[:, i])

        def proj_units(ch, xs, qt_sb):
            units = []
            for dqc in range(4):
                def uq(dqc=dqc):
                    pq = ps_w.tile([128, TCH], F32, tag="ps_w", name="pq")
                    for c in range(NDM):
                        nc.tensor.matmul(
                            pq, lhsT=wq_sb[:, c, dqc * 128:(dqc + 1) * 128],
                            rhs=xs[:, c, :], start=(c == 0), stop=(c == NDM - 1))
                    nc.vector.tensor_copy(out=qt_sb[:, dqc, :], in_=pq)
                units.append(uq)
                def uk(dqc=dqc):
                    pk = ps_w.tile([128, TCH], F32, tag="ps_w", name="pk")
                    for c in range(NDM):
                        nc.tensor.matmul(
                            pk, lhsT=wk_sb[:, c, dqc * 128:(dqc + 1) * 128],
                            rhs=xs[:, c, :], start=(c == 0), stop=(c == NDM - 1))
                    nc.vector.tensor_copy(out=kt_tiles[ch][:, dqc, :], in_=pk)
                units.append(uk)
            for tt in range(4):
                def uv(tt=tt):
                    pv = ps_w.tile([128, HG], F32, tag="ps_w", name="pv")
                    for c in range(NDM):
                        nc.tensor.matmul(
                            pv, lhsT=xs[:, c, tt * 128:(tt + 1) * 128],
                            rhs=wv_sb[:, c, :], start=(c == 0), stop=(c == NDM - 1))
                    nc.vector.tensor_copy(
                        out=v_tiles[ch][:, tt, :, 0:D_HEAD],
                        in_=pv.rearrange("p (h d) -> p h d", h=H))
                units.append(uv)
            return units

        def outproj_units(ch, zt_sb):
            units = []
            q0 = ch * TCH
            for tt in range(4):
                def uo(tt=tt):
                    o_sb = ou_pool.tile([128, D_MODEL], F32, name="o_sb")
                    for dc in range(2):
                        po = ps_w.tile([128, 512], F32, tag="ps_w", name="po")
                        for kc in range(4):
                            nc.tensor.matmul(
                                po, lhsT=zt_sb[:, kc, tt * 128:(tt + 1) * 128],
                                rhs=wo_sb[:, kc, dc * 512:(dc + 1) * 512],
                                start=(kc == 0), stop=(kc == 3))
                        nc.vector.tensor_copy(
                            out=o_sb[:, dc * 512:(dc + 1) * 512], in_=po)
                    r0 = q0 + tt * 128
                    nc.sync.dma_start(out=out.ap()[r0:r0 + 128, :], in_=o_sb)
                units.append(uo)
            return units

        def attention_units(ch, qt_sb, zt_sb):
            """Units: per head-pair, the kb2 iterations (2 k-blocks per score
            psum tile, one merged exp) then the two divisions."""
            units = []
            nkb = 4 * ch + 4
            state = {}
            for hp in range(4):
                def u_alloc(hp=hp):
                    state[hp] = [ps_u.tile([D_HEAD + 1, TCH], F32, name="u_ps",
                                           tag="u_ps") for _ in range(2)]
                units.append(u_alloc)
                for kb2 in range(nkb // 2):
                    def u_kb2(hp=hp, kb2=kb2):
                        u_ps = state[hp]
                        kba, kbb = 2 * kb2, 2 * kb2 + 1
                        ja, jb = kba - 4 * ch, kbb - 4 * ch
                        ca = 128 * ja if ja > 0 else 0
                        cb = 128 * jb if jb > 0 else 0
                        kt_a = kt_tiles[kba // 4]
                        kt_b = kt_tiles[kbb // 4]
                        oa, ob = (kba % 4) * 128, (kbb % 4) * 128
                        s2 = [ps_s.tile([128, 2, TCH], F32, name="s2",
                                        tag="s2") for _ in range(2)]
                        # adjacent (0,0)/(64,0) MMs run concurrently on the PE
                        for par in range(2):
                            p0, p1 = 64 * par, 64 * par + 64
                            nc.tensor.matmul(
                                s2[par][:, 0, ca:],
                                lhsT=kt_a[p0:p1, hp, oa:oa + 128],
                                rhs=qt_sb[p0:p1, hp, ca:],
                                start=True, stop=True,
                                tile_position=(64 * par, 0))
                        for par in range(2):
                            p0, p1 = 64 * par, 64 * par + 64
                            nc.tensor.matmul(
                                s2[par][:, 1, cb:],
                                lhsT=kt_b[p0:p1, hp, ob:ob + 128],
                                rhs=qt_sb[p0:p1, hp, cb:],
                                start=True, stop=True,
                                tile_position=(64 * par, 0))
                        ets = []
                        for par in range(2):
                            et = et_pool.tile([128, 2, TCH], BF16, name="et",
                                              tag="et")
                            s2f = s2[par].rearrange("p a b -> p (a b)")
                            etf = et.rearrange("p a b -> p (a b)")
                            nc.scalar.activation(out=etf[:, ca:],
                                                 in_=s2f[:, ca:],
                                                 func=AF.Exp, scale=0.125)
                            if ja >= 0:
                                nc.vector.tensor_mul(et[:, 0, ca:ca + 128],
                                                     et[:, 0, ca:ca + 128],
                                                     tri_sb)
                            if jb >= 0:
                                nc.vector.tensor_mul(et[:, 1, cb:cb + 128],
                                                     et[:, 1, cb:cb + 128],
                                                     tri_sb)
                            ets.append(et)
                        for par in range(2):
                            h = 2 * hp + par
                            nc.tensor.matmul(
                                u_ps[par][:, ca:],
                                lhsT=v_tiles[kba // 4][:, kba % 4, h, :],
                                rhs=ets[par][:, 0, ca:],
                                start=(kba == 0), stop=False)
                            nc.tensor.matmul(
                                u_ps[par][:, cb:],
                                lhsT=v_tiles[kbb // 4][:, kbb % 4, h, :],
                                rhs=ets[par][:, 1, cb:],
                                start=False, stop=(kbb == nkb - 1))
                    units.append(u_kb2)
                def u_div(hp=hp):
                    u_ps = state[hp]
                    for par in range(2):
                        # zt = U[0:64] / D (D = U row 64): 1/D = exp(-ln D) on
                        # ACT, broadcast over 64 partitions with a K=1 matmul.
                        lnd = sm_pool.tile([1, TCH], F32, name="lnd")
                        nc.scalar.activation(
                            out=lnd, in_=u_ps[par][D_HEAD:D_HEAD + 1, :],
                            func=AF.Ln)
                        rcp = sm_pool.tile([1, TCH], F32R, name="rcp")
                        nc.scalar.activation(out=rcp, in_=lnd, func=AF.Exp,
                                             scale=-1.0)
                        db_ps = ps_s.tile([64, TCH], F32, tag="s2", name="db_ps")
                        nc.tensor.matmul(db_ps, lhsT=ones_sb, rhs=rcp,
                                         start=True, stop=True)
                        rb = rb_pool.tile([64, TCH], F32)
                        nc.vector.tensor_copy(out=rb, in_=db_ps)
                        nc.vector.tensor_mul(
                            zt_sb[64 * par:64 * par + 64, hp, :],
                            u_ps[par][0:D_HEAD, :], rb)
                units.append(u_div)
            return units

        # ---- software-pipelined emission ----
        # chunk 0 projections up front; then for each chunk, its attention
        # units interleaved with (prev chunk's out-proj + next chunk's proj).
        qt_tiles = [None] * NCH
        xs_tiles = [xs0] + [None] * (NCH - 1)
        zt_tiles = [None] * NCH

        def stage_proj(ch):
            if ch >= NCH:
                return []
            if ch > 0:
                xs_tiles[ch] = xs_pool.tile([128, NDM, TCH], F32R, name="xs",
                                            tag="xs")
                nc.sync.dma_start(
                    out=xs_tiles[ch],
                    in_=xT_r[:, :, ch * TCH:(ch + 1) * TCH])
            qt_tiles[ch] = qt_pool.tile([128, 4, TCH], BF16, name="qt",
                                        tag="qt")
            return proj_units(ch, xs_tiles[ch], qt_tiles[ch])

        for u in stage_proj(0):
            u()
        # fill schedule: att0 | proj1+proj2, att1 | outproj0+proj3,
        # att2 | outproj1, att3 | outproj2, then outproj3.
        for ch in range(NCH):
            zt_tiles[ch] = zt_pool.tile([128, 4, TCH], F32R, name="zt",
                                        tag="zt")
            au = attention_units(ch, qt_tiles[ch], zt_tiles[ch])
            fill = []
            if ch >= 1:
                fill += outproj_units(ch - 1, zt_tiles[ch - 1])
            if ch == 0:
                fill += stage_proj(1) + stage_proj(2)
            elif ch == 1:
                fill += stage_proj(3)
            k = 0
            for i, a in enumerate(au):
                a()
                want = (i + 1) * len(fill) // len(au)
                while k < want:
                    fill[k]()
                    k += 1
            while k < len(fill):
                fill[k]()
                k += 1
        for u in outproj_units(NCH - 1, zt_tiles[NCH - 1]):
            u()

    _split_multi_waits(nc)
    return nc


_NC_CACHE = None


def _get_nc():
    global _NC_CACHE
    if _NC_CACHE is None:
        _NC_CACHE = _build()
    return _NC_CACHE


def _make_in_maps(x, W_Q, W_K, W_V, W_O):
    x = np.asarray(x, dtype=np.float32)
    W_Q = np.asarray(W_Q, dtype=np.float32)
    W_K = np.asarray(W_K, dtype=np.float32)
    W_V = np.asarray(W_V, dtype=np.float32)
    W_O = np.asarray(W_O, dtype=np.float32)

    import ml_dtypes
    tri = np.triu(np.ones((128, 128), dtype=ml_dtypes.bfloat16))  # col >= row
    ones1 = np.ones((1, 64), dtype=np.float32)
    vones = np.ones((128, T // 128, H, 1), dtype=ml_dtypes.bfloat16)

    in_maps = []
    for core in range(8):
        b, g = core // 2, core % 2
        cs = slice(g * HG, (g + 1) * HG)
        in_maps.append({
            "xT": np.ascontiguousarray(x[b].T),
            "wq": np.ascontiguousarray(W_Q[:, cs]),
            "wk": np.ascontiguousarray(W_K[:, cs]),
            "wv": np.ascontiguousarray(W_V[:, cs]),
            "wo": np.ascontiguousarray(W_O[cs, :]),
            "tri": tri, "ones1": ones1, "vones": vones,
        })
    return in_maps


def kernel(x, W_Q, W_K, W_V, W_O):
    in_maps = _make_in_maps(x, W_Q, W_K, W_V, W_O)
    nc = _get_nc()
    res = run_bass_kernel_spmd(nc, in_maps, core_ids=list(range(8)))
    outs = [res.results[c]["out"] for c in range(8)]
    full = np.stack([outs[2 * b] + outs[2 * b + 1] for b in range(B)], axis=0)
    return full



# revision 8
# speedup vs baseline: 1.3284x; 1.3284x over previous
"""Causal multi-head attention on 8 TRN2 NeuronCores.

Problem: B=4, T=2048, d_model=1024, 16 heads x 64. out = softmax(causal(QK^T)/8) V Wo.

Sharding (tensor-parallel heads x data-parallel batch):
  core c -> batch b = c//2, head group g = c%2 (8 heads each).
  Each core computes a partial output  z_g[b] @ Wo[g] : [2048, 1024];
  host sums the two head-group partials per batch.

Per-core kernel (all matmuls in fp32r = fp32 data at full PE rate):
  - host passes x[b]^T (so d_model is the SBUF partition dim everywhere)
  - fused single pass over 4 query chunks of 512:
      proj (Q^T,K^T,V for the chunk) -> causal flash-style attention over
      k-blocks of 128 (scores pairs of heads via tile_position row-split,
      exp on ACT, V-augmented-with-ones matmul accumulates both z^T and the
      softmax denominator) -> divide -> output projection -> DMA out.
"""
import numpy as np

import concourse.bass as bass
import concourse.tile as tile
import concourse.mybir as mybir
from concourse.vector_clock import ScopedClock
from concourse.bass_utils import run_bass_kernel_spmd

D_MODEL = 1024
D_HEAD = 64
B = 4
T = 2048
H = 8              # heads per core
HG = H * D_HEAD    # 512 head-dim columns per core
TCH = 512          # q/t chunk
NCH = T // TCH     # 4
NDM = D_MODEL // 128  # 8 d_model chunks

F32R = mybir.dt.float32r
F32 = mybir.dt.float32
BF16 = mybir.dt.bfloat16
AF = mybir.ActivationFunctionType


class _TC(tile.TileContext):
    """TileContext whose tail drain carries no sem waits (this walrus build
    rejects >1 sync wait per instruction and any wait on a Drain)."""

    def _drain_and_barrier(self, tick_clock, wait_clock):
        drain_inst = self.nc.sync.drain()
        wait_clock.add_sem_waits(
            drain_inst.ins, ScopedClock({None: tick_clock.global_clock})
        )
        si = drain_inst.ins.sync_info
        waits = list(si.on_wait) if si is not None else []
        if waits:
            drain_inst.ins.sync_info = mybir.SyncInfo(
                on_wait=[], on_update=list(si.on_update)
            )
            for w in waits:
                nop = self.nc.sync.nop(nofuse=True)
                nop.ins.sync_info = mybir.SyncInfo(on_wait=[w], on_update=[])
        self.nc.all_engine_barrier()
        popped = self.nc._tile_sem_poison_stack.pop()
        assert popped is self._sem_poison
        self.nc.clear_and_free_semaphores(list(self.sems.allocated().values()))
        self.nc.all_engine_barrier()


def _split_multi_waits(nc):
    """Move all-but-one sem wait of every instruction onto same-engine NOPs."""
    cnt = 0
    for f in nc.m.functions:
        for b in f.blocks:
            new = []
            for inst in b.instructions:
                si = inst.sync_info
                if si is not None and si.on_wait is not None:
                    waits = list(si.on_wait)
                    max_keep = 0 if inst.opcode == "Drain" else 1
                    if len(waits) > max_keep:
                        keep = waits[len(waits) - max_keep:] if max_keep else []
                        spill = waits[: len(waits) - max_keep]
                        for w in spill:
                            nop = mybir.InstNoOp(
                                name=f"I-wsplit-{cnt}", engine=inst.engine,
                                ins=[], outs=[],
                            )
                            nop.sync_info = mybir.SyncInfo(
                                on_wait=[w], on_update=[]
                            )
                            new.append(nop)
                            cnt += 1
                        inst.sync_info = mybir.SyncInfo(
                            on_wait=keep, on_update=list(si.on_update)
                        )
                new.append(inst)
            b.instructions = new
    return cnt


def _build():
    nc = bass.Bass("TRN2", target_bir_lowering=False)
    xT = nc.dram_tensor("xT", (D_MODEL, T), BF16, kind="ExternalInput")
    wq = nc.dram_tensor("wq", (D_MODEL, HG), BF16, kind="ExternalInput")
    wk = nc.dram_tensor("wk", (D_MODEL, HG), BF16, kind="ExternalInput")
    wv = nc.dram_tensor("wv", (D_MODEL, HG), BF16, kind="ExternalInput")
    wo = nc.dram_tensor("wo", (HG, D_MODEL), BF16, kind="ExternalInput")
    tri = nc.dram_tensor("tri", (128, 128), BF16, kind="ExternalInput")
    ones1 = nc.dram_tensor("ones1", (1, 64), BF16, kind="ExternalInput")
    vones = nc.dram_tensor("vones", (128, T // 128, H, 1), BF16,
                           kind="ExternalInput")
    out = nc.dram_tensor("out", (T, D_MODEL), F32, kind="ExternalOutput")

    from contextlib import ExitStack
    with _TC(nc) as tc, ExitStack() as ctx:
        consts = ctx.enter_context(tc.tile_pool(name="consts", bufs=1))
        xs_pool = ctx.enter_context(tc.tile_pool(name="xs", bufs=3))
        kt_pool = ctx.enter_context(tc.tile_pool(name="kt", bufs=1))
        v_pool = ctx.enter_context(tc.tile_pool(name="v", bufs=1))
        qt_pool = ctx.enter_context(tc.tile_pool(name="qt", bufs=3))
        zt_pool = ctx.enter_context(tc.tile_pool(name="zt", bufs=2))
        et_pool = ctx.enter_context(tc.tile_pool(name="et", bufs=4))
        sm_pool = ctx.enter_context(tc.tile_pool(name="sm", bufs=2))
        rb_pool = ctx.enter_context(tc.tile_pool(name="rb", bufs=2))
        ou_pool = ctx.enter_context(tc.tile_pool(name="ou", bufs=2))
        ps_s = ctx.enter_context(tc.tile_pool(name="ps_s", bufs=2, space="PSUM"))
        ps_u = ctx.enter_context(tc.tile_pool(name="ps_u", bufs=2, space="PSUM"))
        ps_w = ctx.enter_context(tc.tile_pool(name="ps_w", bufs=2, space="PSUM"))

        xT_r = xT.ap().rearrange("(c p) t -> p c t", p=128)

        # resident weights / constants (wq/wk + first x chunk lead: they gate
        # the first matmuls; wo is not needed until the first out-proj)
        wq_sb = consts.tile([128, NDM, HG], BF16)
        xs0 = xs_pool.tile([128, NDM, TCH], BF16, name="xs", tag="xs")
        wk_sb = consts.tile([128, NDM, HG], BF16)
        wv_sb = consts.tile([128, NDM, HG], BF16)
        wq_r = wq.ap().rearrange("(c p) n -> p c n", p=128)
        wk_r = wk.ap().rearrange("(c p) n -> p c n", p=128)
        for c in range(NDM):
            nc.sync.dma_start(out=wq_sb[:, c, :], in_=wq_r[:, c, :])
            nc.sync.dma_start(out=wk_sb[:, c, :], in_=wk_r[:, c, :])
            nc.sync.dma_start(out=xs0[:, c, :], in_=xT_r[:, c, 0:TCH])
        nc.sync.dma_start(out=wv_sb, in_=wv.ap().rearrange("(c p) n -> p c n", p=128))
        tri_sb = consts.tile([128, 128], BF16)
        nc.sync.dma_start(out=tri_sb, in_=tri.ap())
        ones_sb = consts.tile([1, 64], BF16)
        nc.sync.dma_start(out=ones_sb, in_=ones1.ap())
        wo_sb = consts.tile([128, HG // 128, D_MODEL], BF16)
        nc.sync.dma_start(out=wo_sb, in_=wo.ap().rearrange("(c p) n -> p c n", p=128))
        # per-chunk K^T tiles [pair-packed 128, pair, t-in-chunk] and V tiles
        # (V has a ones column so row 64 of U accumulates the denominator)
        kt_tiles = [kt_pool.tile([128, 4, TCH], BF16, name=f"kt{i}", tag=f"kt{i}")
                    for i in range(NCH)]
        v_tiles = [v_pool.tile([128, 4, H, D_HEAD + 1], BF16, name=f"v{i}",
                               tag=f"v{i}") for i in range(NCH)]
        vo_r = vones.ap().rearrange("p (a b) h o -> p a b h o", b=4)
        for i in range(NCH):
            nc.sync.dma_start(out=v_tiles[i][:, :, :, D_HEAD:], in_=vo_r[:, i])

        def proj_units(ch, xs, qt_sb):
            units = []
            for dqc in range(4):
                def uq(dqc=dqc):
                    pq = ps_w.tile([128, TCH], F32, tag="ps_w", name="pq")
                    for c in range(NDM):
                        nc.tensor.matmul(
                            pq, lhsT=wq_sb[:, c, dqc * 128:(dqc + 1) * 128],
                            rhs=xs[:, c, :], start=(c == 0), stop=(c == NDM - 1))
                    nc.vector.tensor_copy(out=qt_sb[:, dqc, :], in_=pq)
                units.append(uq)
                def uk(dqc=dqc):
                    pk = ps_w.tile([128, TCH], F32, tag="ps_w", name="pk")
                    for c in range(NDM):
                        nc.tensor.matmul(
                            pk, lhsT=wk_sb[:, c, dqc * 128:(dqc + 1) * 128],
                            rhs=xs[:, c, :], start=(c == 0), stop=(c == NDM - 1))
                    nc.vector.tensor_copy(out=kt_tiles[ch][:, dqc, :], in_=pk)
                units.append(uk)
            for tt in range(4):
                def uv(tt=tt):
                    pv = ps_w.tile([128, HG], F32, tag="ps_w", name="pv")
                    for c in range(NDM):
                        nc.tensor.matmul(
                            pv, lhsT=xs[:, c, tt * 128:(tt + 1) * 128],
                            rhs=wv_sb[:, c, :], start=(c == 0), stop=(c == NDM - 1))
                    nc.vector.tensor_copy(
                        out=v_tiles[ch][:, tt, :, 0:D_HEAD],
                        in_=pv.rearrange("p (h d) -> p h d", h=H))
                units.append(uv)
            return units

        def outproj_units(ch, zt_sb):
            units = []
            q0 = ch * TCH
            for tt in range(4):
                def uo(tt=tt):
                    o_sb = ou_pool.tile([128, D_MODEL], F32, name="o_sb")
                    for dc in range(2):
                        po = ps_w.tile([128, 512], F32, tag="ps_w", name="po")
                        for kc in range(4):
                            nc.tensor.matmul(
                                po, lhsT=zt_sb[:, kc, tt * 128:(tt + 1) * 128],
                                rhs=wo_sb[:, kc, dc * 512:(dc + 1) * 512],
                                start=(kc == 0), stop=(kc == 3))
                        nc.vector.tensor_copy(
                            out=o_sb[:, dc * 512:(dc + 1) * 512], in_=po)
                    r0 = q0 + tt * 128
                    nc.sync.dma_start(out=out.ap()[r0:r0 + 128, :], in_=o_sb)
                units.append(uo)
            return units

        def attention_units(ch, qt_sb, zt_sb):
            """Units: per head-pair, the kb2 iterations (2 k-blocks per score
            psum tile, one merged exp) then the two divisions."""
            units = []
            nkb = 4 * ch + 4
            state = {}
            for hp in range(4):
                def u_alloc(hp=hp):
                    state[hp] = [ps_u.tile([D_HEAD + 1, TCH], F32, name="u_ps",
                                           tag="u_ps") for _ in range(2)]
                units.append(u_alloc)
                for kb2 in range(nkb // 2):
                    def u_kb2(hp=hp, kb2=kb2):
                        u_ps = state[hp]
                        kba, kbb = 2 * kb2, 2 * kb2 + 1
                        ja, jb = kba - 4 * ch, kbb - 4 * ch
                        ca = 128 * ja if ja > 0 else 0
                        cb = 128 * jb if jb > 0 else 0
                        kt_a = kt_tiles[kba // 4]
                        kt_b = kt_tiles[kbb // 4]
                        oa, ob = (kba % 4) * 128, (kbb % 4) * 128
                        s2 = [ps_s.tile([128, 2, TCH], F32, name="s2",
                                        tag="s2") for _ in range(2)]
                        # adjacent (0,0)/(64,0) MMs run concurrently on the PE
                        for par in range(2):
                            p0, p1 = 64 * par, 64 * par + 64
                            nc.tensor.matmul(
                                s2[par][:, 0, ca:],
                                lhsT=kt_a[p0:p1, hp, oa:oa + 128],
                                rhs=qt_sb[p0:p1, hp, ca:],
                                start=True, stop=True,
                                tile_position=(64 * par, 0))
                        for par in range(2):
                            p0, p1 = 64 * par, 64 * par + 64
                            nc.tensor.matmul(
                                s2[par][:, 1, cb:],
                                lhsT=kt_b[p0:p1, hp, ob:ob + 128],
                                rhs=qt_sb[p0:p1, hp, cb:],
                                start=True, stop=True,
                                tile_position=(64 * par, 0))
                        ets = []
                        for par in range(2):
                            et = et_pool.tile([128, 2, TCH], BF16, name="et",
                                              tag="et")
                            s2f = s2[par].rearrange("p a b -> p (a b)")
                            etf = et.rearrange("p a b -> p (a b)")
                            nc.scalar.activation(out=etf[:, ca:],
                                                 in_=s2f[:, ca:],
                                                 func=AF.Exp, scale=0.125)
                            if ja >= 0:
                                nc.vector.tensor_mul(et[:, 0, ca:ca + 128],
                                                     et[:, 0, ca:ca + 128],
                                                     tri_sb)
                            if jb >= 0:
                                nc.vector.tensor_mul(et[:, 1, cb:cb + 128],
                                                     et[:, 1, cb:cb + 128],
                                                     tri_sb)
                            ets.append(et)
                        for par in range(2):
                            h = 2 * hp + par
                            nc.tensor.matmul(
                                u_ps[par][:, ca:],
                                lhsT=v_tiles[kba // 4][:, kba % 4, h, :],
                                rhs=ets[par][:, 0, ca:],
                                start=(kba == 0), stop=False)
                            nc.tensor.matmul(
                                u_ps[par][:, cb:],
                                lhsT=v_tiles[kbb // 4][:, kbb % 4, h, :],
                                rhs=ets[par][:, 1, cb:],
                                start=False, stop=(kbb == nkb - 1))
                    units.append(u_kb2)
                def u_div(hp=hp):
                    u_ps = state[hp]
                    for par in range(2):
                        # zt = U[0:64] / D (D = U row 64): 1/D = exp(-ln D) on
                        # ACT, broadcast over 64 partitions with a K=1 matmul.
                        lnd = sm_pool.tile([1, TCH], F32, name="lnd")
                        nc.scalar.activation(
                            out=lnd, in_=u_ps[par][D_HEAD:D_HEAD + 1, :],
                            func=AF.Ln)
                        rcp = sm_pool.tile([1, TCH], BF16, name="rcp")
                        nc.scalar.activation(out=rcp, in_=lnd, func=AF.Exp,
                                             scale=-1.0)
                        db_ps = ps_s.tile([64, TCH], F32, tag="s2", name="db_ps")
                        nc.tensor.matmul(db_ps, lhsT=ones_sb, rhs=rcp,
                                         start=True, stop=True)
                        rb = rb_pool.tile([64, TCH], F32)
                        nc.vector.tensor_copy(out=rb, in_=db_ps)
                        nc.vector.tensor_mul(
                            zt_sb[64 * par:64 * par + 64, hp, :],
                            u_ps[par][0:D_HEAD, :], rb)
                units.append(u_div)
            return units

        # ---- software-pipelined emission ----
        # chunk 0 projections up front; then for each chunk, its attention
        # units interleaved with (prev chunk's out-proj + next chunk's proj).
        qt_tiles = [None] * NCH
        xs_tiles = [xs0] + [None] * (NCH - 1)
        zt_tiles = [None] * NCH

        def stage_proj(ch):
            if ch >= NCH:
                return []
            if ch > 0:
                xs_tiles[ch] = xs_pool.tile([128, NDM, TCH], BF16, name="xs",
                                            tag="xs")
                nc.sync.dma_start(
                    out=xs_tiles[ch],
                    in_=xT_r[:, :, ch * TCH:(ch + 1) * TCH])
            qt_tiles[ch] = qt_pool.tile([128, 4, TCH], BF16, name="qt",
                                        tag="qt")
            return proj_units(ch, xs_tiles[ch], qt_tiles[ch])

        for u in stage_proj(0):
            u()
        # fill schedule: att0 | proj1+proj2, att1 | outproj0+proj3,
        # att2 | outproj1, att3 | outproj2, then outproj3.
        for ch in range(NCH):
            zt_tiles[ch] = zt_pool.tile([128, 4, TCH], BF16, name="zt",
                                        tag="zt")
            au = attention_units(ch, qt_tiles[ch], zt_tiles[ch])
            fill = []
            if ch >= 1:
                fill += outproj_units(ch - 1, zt_tiles[ch - 1])
            if ch == 0:
                fill += stage_proj(1) + stage_proj(2)
            elif ch == 1:
                fill += stage_proj(3)
            k = 0
            for i, a in enumerate(au):
                a()
                want = (i + 1) * len(fill) // len(au)
                while k < want:
                    fill[k]()
                    k += 1
            while k < len(fill):
                fill[k]()
                k += 1
        for u in outproj_units(NCH - 1, zt_tiles[NCH - 1]):
            u()

    _split_multi_waits(nc)
    return nc


_NC_CACHE = None


def _get_nc():
    global _NC_CACHE
    if _NC_CACHE is None:
        _NC_CACHE = _build()
    return _NC_CACHE


def _make_in_maps(x, W_Q, W_K, W_V, W_O):
    x = np.asarray(x, dtype=np.float32)
    W_Q = np.asarray(W_Q, dtype=np.float32)
    W_K = np.asarray(W_K, dtype=np.float32)
    W_V = np.asarray(W_V, dtype=np.float32)
    W_O = np.asarray(W_O, dtype=np.float32)

    import ml_dtypes
    bf = ml_dtypes.bfloat16
    tri = np.triu(np.ones((128, 128), dtype=bf))  # col >= row
    ones1 = np.ones((1, 64), dtype=bf)
    vones = np.ones((128, T // 128, H, 1), dtype=bf)

    in_maps = []
    for core in range(8):
        b, g = core // 2, core % 2
        cs = slice(g * HG, (g + 1) * HG)
        in_maps.append({
            "xT": np.ascontiguousarray(x[b].T).astype(bf),
            "wq": np.ascontiguousarray(W_Q[:, cs]).astype(bf),
            "wk": np.ascontiguousarray(W_K[:, cs]).astype(bf),
            "wv": np.ascontiguousarray(W_V[:, cs]).astype(bf),
            "wo": np.ascontiguousarray(W_O[cs, :]).astype(bf),
            "tri": tri, "ones1": ones1, "vones": vones,
        })
    return in_maps


def kernel(x, W_Q, W_K, W_V, W_O):
    in_maps = _make_in_maps(x, W_Q, W_K, W_V, W_O)
    nc = _get_nc()
    res = run_bass_kernel_spmd(nc, in_maps, core_ids=list(range(8)))
    outs = [res.results[c]["out"] for c in range(8)]
    full = np.stack([outs[2 * b] + outs[2 * b + 1] for b in range(B)], axis=0)
    return full

